# revision 15
# baseline (speedup 1.0000x reference)
"""Trainium2 Bass kernel for nn_AttnGCN (3-layer GATv2 + BN + FC over a 30K-node graph).

Strategy (8 NeuronCores, SPMD):
- Edges (with self-loops) are sorted by destination on the host and sharded
  across cores by contiguous destination-node ranges (3750 nodes/core).
- Within a core, edges are grouped into 128-node destination windows and
  padded to a chunk schedule (128 edges/chunk) that is identical across
  cores, so one instruction stream serves all 8 cores.
- Node projections xl = x@Wl (all nodes, replicated) are written to a DRAM
  table; per-chunk the kernel gathers xl[src] via indirect DMA, expands
  xr[dst]+eh via a PE matmul against a 0/1 indicator matrix, computes the
  GATv2 logit with a fused multiply-reduce, and aggregates
  sum_e I[e,n]*q_e*xl[src_e] (and s_n = sum q) with PE matmuls into PSUM.
  Softmax normalization folds into a per-node scale after aggregation, so
  attention weights are never materialized and xl is gathered once.
- BatchNorm statistics are computed redundantly on every core from the
  AllGathered feature table (layers 1,2) or via a tiny AllReduce (final
  layer), then fused into the next matmul's input transform.
"""

import sys

sys.path.insert(0, "/opt/trn_rl_repo")

import numpy as np
import ml_dtypes

import concourse.bacc as bacc
import concourse.bass as bass
import concourse.mybir as mybir
import concourse.tile as tile
from concourse.tile import TileContext

F32 = mybir.dt.float32
BF16 = mybir.dt.bfloat16
I32 = mybir.dt.int32
NPF32 = np.float32
NPBF16 = np.dtype(ml_dtypes.bfloat16)

AX = mybir.AxisListType
OP = mybir.AluOpType
ACT = mybir.ActivationFunctionType

NCORES = 8
H = 2
C = 128
HC = H * C  # 256
WIN = 128


class Cfg:
    def __init__(self, N, E, IN=6, OUT=5):
        self.N = N
        self.E = E
        self.IN = IN
        self.OUT = OUT
        self.NPC = N // NCORES
        self.NW = (self.NPC + WIN - 1) // WIN
        assert self.E % 128 == 0
        self.EW_COLS = self.E // 128


def preprocess(cfg, edge_index, edge_weight):
    """Host-side integer/graph-structure preprocessing. Returns per-core arrays."""
    N, E = cfg.N, cfg.E
    NPC, NW = cfg.NPC, cfg.NW
    src = np.concatenate([edge_index[0], np.arange(N)]).astype(np.int64)
    dst = np.concatenate([edge_index[1], np.arange(N)]).astype(np.int64)
    ew = np.concatenate(
        [edge_weight.reshape(-1).astype(np.float64), np.zeros(N)]
    ).astype(NPF32)
    mk = np.concatenate([np.zeros(E, NPF32), np.ones(N, NPF32)]).astype(NPF32)
    order = np.argsort(dst, kind="stable")
    src, dst, ew, mk = src[order], dst[order], ew[order], mk[order]
    deg = np.bincount(dst, minlength=N).astype(NPF32)

    core = dst // NPC
    rel = dst - core * NPC
    win = rel // WIN
    dstrel = (rel - win * WIN).astype(NPF32)

    counts = np.zeros((NCORES, NW), np.int64)
    np.add.at(counts, (core, win), 1)
    nch = np.maximum(1, -(-counts // 128)).max(axis=0)  # [NW] per-window chunks
    totch = int(nch.sum())
    ch_off = np.zeros(NW + 1, np.int64)
    ch_off[1:] = np.cumsum(nch)

    src_cols = np.zeros((NCORES, 128, totch), np.int32)
    dst_cols = np.full((NCORES, 128, totch), -1.0, NPF32)
    eam = np.zeros((NCORES, 2, totch * 128), NPF32)
    for k in range(NCORES):
        sel = core == k
        s_k, d_k, e_k, m_k, w_k = src[sel], dstrel[sel], ew[sel], mk[sel], win[sel]
        wsort = np.argsort(w_k, kind="stable")
        s_k, d_k, e_k, m_k, w_k = (
            s_k[wsort],
            d_k[wsort],
            e_k[wsort],
            m_k[wsort],
            w_k[wsort],
        )
        wcnt = np.bincount(w_k, minlength=NW)
        pos = 0
        for w in range(NW):
            cnt = int(wcnt[w])
            c0 = int(ch_off[w])
            ncw = int(nch[w])
            slots = ncw * 128
            sc = np.zeros(slots, np.int32)
            dc = np.full(slots, -1.0, NPF32)
            ec = np.zeros(slots, NPF32)
            mc = np.zeros(slots, NPF32)
            sc[:cnt] = s_k[pos : pos + cnt]
            dc[:cnt] = d_k[pos : pos + cnt]
            ec[:cnt] = e_k[pos : pos + cnt]
            mc[:cnt] = m_k[pos : pos + cnt]
            pos += cnt
            src_cols[k, :, c0 : c0 + ncw] = sc.reshape(ncw, 128).T
            dst_cols[k, :, c0 : c0 + ncw] = dc.reshape(ncw, 128).T
            eam[k, 0, c0 * 128 : (c0 + ncw) * 128] = ec
            eam[k, 1, c0 * 128 : (c0 + ncw) * 128] = mc

    r2d = np.ones((NCORES, 128, NW), NPF32)
    for k in range(NCORES):
        for w in range(NW):
            nn = min(WIN, NPC - w * WIN)
            ids = k * NPC + w * WIN + np.arange(nn)
            r2d[k, :nn, w] = 1.0 / (2.0 * deg[ids])

    return dict(
        nch=[int(x) for x in nch],
        ch_off=[int(x) for x in ch_off],
        totch=totch,
        src_cols=src_cols,
        dst_cols=dst_cols,
        eam=eam,
        r2d=r2d,
    )


def build_program(cfg, nch, ch_off, totch, lmax=3, do_edge=True, do_cc=True, do_fc=True, eparts=5):
    """Build the SPMD Bass program. Returns (nc, input names are fixed)."""
    N, E, IN, OUT = cfg.N, cfg.E, cfg.IN, cfg.OUT
    NPC, NW, EW_COLS = cfg.NPC, cfg.NW, cfg.EW_COLS

    nc = bacc.Bacc("TRN2", target_bir_lowering=False, debug=False, num_devices=NCORES)

    def din(name, shape, dt):
        return nc.declare_dram_parameter(name, list(shape), dt, isOutput=False)

    # --- inputs ---
    hT = din("hT", [IN, N], BF16)
    hTloc = din("hTloc", [IN, NPC], BF16)
    Wl, bl, Wr, br, We, attrep, brep, gcol, btcol = [], [], [], [], [], [], [], [], []
    for l in range(3):
        d = IN if l == 0 else C
        Wl.append(din(f"Wl{l}", [d, HC], BF16))
        bl.append(din(f"bl{l}", [1, HC], BF16))
        Wr.append(din(f"Wr{l}", [d, HC], BF16))
        br.append(din(f"br{l}", [1, HC], BF16))
        We.append(din(f"We{l}", [1, HC], BF16))
        attrep.append(din(f"attrep{l}", [128, HC], F32))
        brep.append(din(f"brep{l}", [128, C], F32))
        gcol.append(din(f"g{l}", [128, 1], F32))
        btcol.append(din(f"bt{l}", [128, 1], F32))
    fcW = din("fcW", [C, OUT], BF16)
    fcb = din("fcb", [1, OUT], BF16)
    ew = din("ew", [128, EW_COLS], F32)
    iota = din("iota", [128, 128], F32)
    identbf = din("identbf", [128, 128], BF16)
    onesrow = din("onesrow", [1, 128], BF16)
    onescol = din("onescol", [128, 1], BF16)
    lastmask = din("lastmask", [128, 1], F32)
    r2d_in = din("r2d", [128, NW], F32)
    srccols = din("srccols", [128, totch], I32)
    dstcols = din("dstcols", [128, totch], F32)
    eam_in = din("eam", [2, totch * 128], BF16)

    out_ext = nc.declare_dram_parameter("out", [NPC, OUT], F32, isOutput=True)

    # --- internal DRAM ---
    xl_dram = nc.dram_tensor("xl_tab", [N, HC], BF16)
    ftl_dram = nc.dram_tensor("ftl", [C, NPC], BF16)
    ftg_dram = nc.dram_tensor("ftg", [NCORES * C, NPC], BF16, addr_space="Shared")
    arin = nc.dram_tensor("arin", [128, 2], F32)
    arout = nc.dram_tensor("arout", [128, 2], F32, addr_space="Shared")

    RG = [list(range(NCORES))]

    with TileContext(nc) as tc:
        const = tc.alloc_tile_pool(name="const", bufs=1)
        lw = tc.alloc_tile_pool(name="lw", bufs=2)  # per-layer weights
        persist = tc.alloc_tile_pool(name="persist", bufs=2)
        bnp = tc.alloc_tile_pool(name="bnp", bufs=2)

        # ---- load constants ----
        iota_t = const.tile([128, 128], F32)
        nc.sync.dma_start(out=iota_t[:], in_=iota[:])
        ident_t = const.tile([128, 128], BF16)
        nc.sync.dma_start(out=ident_t[:], in_=identbf[:])
        onesrow_t = const.tile([1, 128], BF16)
        nc.sync.dma_start(out=onesrow_t[:], in_=onesrow[:])
        onescol_t = const.tile([128, 1], BF16)
        nc.sync.dma_start(out=onescol_t[:], in_=onescol[:])
        lastmask_t = const.tile([128, 1], F32)
        nc.sync.dma_start(out=lastmask_t[:], in_=lastmask[:])
        r2d_t = const.tile([128, NW], F32)
        nc.sync.dma_start(out=r2d_t[:], in_=r2d_in[:])
        src_t = const.tile([128, totch], I32)
        nc.sync.dma_start(out=src_t[:], in_=srccols[:])
        dst_t = const.tile([128, totch], F32)
        nc.sync.dma_start(out=dst_t[:], in_=dstcols[:])
        fcW_t = const.tile([C, OUT], BF16)
        nc.sync.dma_start(out=fcW_t[:], in_=fcW[:])
        fcb_t = const.tile([1, OUT], BF16)
        nc.sync.dma_start(out=fcb_t[:], in_=fcb[:])

        # ---- mean of edge weights (device) ----
        with (
            tc.tile_pool(name="mseb", bufs=2) as mseb,
            tc.tile_pool(name="msps", bufs=1, space="PSUM") as msps,
        ):
            ew_t = mseb.tile([128, EW_COLS], F32)
            nc.sync.dma_start(out=ew_t[:], in_=ew[:])
            colsum = mseb.tile([128, 1], F32)
            nc.vector.reduce_sum(out=colsum[:], in_=ew_t[:], axis=AX.X)
            colsum_bf = mseb.tile([128, 1], BF16)
            nc.vector.tensor_copy(out=colsum_bf[:], in_=colsum[:])
            mps = msps.tile([1, 1], F32)
            nc.tensor.matmul(
                out=mps[:], lhsT=colsum_bf[:], rhs=onescol_t[:], start=True, stop=True
            )
            mean_t = const.tile([1, 1], F32)
            nc.scalar.activation(
                out=mean_t[:], in_=mps[:], func=ACT.Copy, scale=1.0 / E
            )

        # helper: emit BN param computation from S1/S2 [128,1] f32 APs
        def emit_bn_params(S1, S2, l):
            g_t = bnp.tile([128, 1], F32, tag="bn_g")
            nc.sync.dma_start(out=g_t[:], in_=gcol[l][:])
            bt_t = bnp.tile([128, 1], F32, tag="bn_bt")
            nc.sync.dma_start(out=bt_t[:], in_=btcol[l][:])
            mean = bnp.tile([128, 1], F32, tag="bn_mean")
            nc.scalar.activation(out=mean[:], in_=S1, func=ACT.Copy, scale=1.0 / N)
            m2 = bnp.tile([128, 1], F32, tag="bn_m2")
            nc.vector.tensor_mul(out=m2[:], in0=mean[:], in1=mean[:])
            var = bnp.tile([128, 1], F32, tag="bn_var")
            nc.vector.tensor_scalar(
                out=var[:],
                in0=S2,
                scalar1=1.0 / N,
                scalar2=m2[:, :1],
                op0=OP.mult,
                op1=OP.subtract,
            )
            eps = bnp.tile([128, 1], F32, tag="bn_eps")
            nc.vector.memset(eps[:], 1e-5)
            sq = bnp.tile([128, 1], F32, tag="bn_sq")
            nc.scalar.activation(out=sq[:], in_=var[:], func=ACT.Sqrt, bias=eps[:, :1])
            rstd = bnp.tile([128, 1], F32, tag="bn_rstd")
            nc.vector.reciprocal(out=rstd[:], in_=sq[:])
            sc = bnp.tile([128, 1], F32, tag="bn_sc")
            nc.vector.tensor_mul(out=sc[:], in0=g_t[:], in1=rstd[:])
            tmp = bnp.tile([128, 1], F32, tag="bn_tmp")
            nc.vector.tensor_mul(out=tmp[:], in0=mean[:], in1=sc[:])
            bi = bnp.tile([128, 1], F32, tag="bn_bi")
            nc.vector.tensor_sub(out=bi[:], in0=bt_t[:], in1=tmp[:])
            sc001 = bnp.tile([128, 1], F32, tag="bn_sc001")
            nc.vector.tensor_scalar_mul(out=sc001[:], in0=sc[:], scalar1=0.01)
            bi001 = bnp.tile([128, 1], F32, tag="bn_bi001")
            nc.vector.tensor_scalar_mul(out=bi001[:], in0=bi[:], scalar1=0.01)
            return sc, bi, sc001, bi001

        ftl_sb_prev = None  # SBUF featT of previous layer (this core's nodes)

        for l in range(lmax):
            d = IN if l == 0 else C

            # ---- load layer weights ----
            Wl_t = lw.tile([d, HC], BF16, tag="Wl")
            nc.sync.dma_start(out=Wl_t[:], in_=Wl[l][:])
            Wr_t = lw.tile([d, HC], BF16, tag="Wr")
            nc.sync.dma_start(out=Wr_t[:], in_=Wr[l][:])
            bl_t = lw.tile([1, HC], BF16, tag="bl")
            nc.sync.dma_start(out=bl_t[:], in_=bl[l][:])
            br_t = lw.tile([1, HC], BF16, tag="br")
            nc.sync.dma_start(out=br_t[:], in_=br[l][:])
            att_t = lw.tile([128, HC], F32, tag="att")
            nc.sync.dma_start(out=att_t[:], in_=attrep[l][:])
            brep_t = lw.tile([128, C], F32, tag="brep")
            nc.sync.dma_start(out=brep_t[:], in_=brep[l][:])
            # WeM: row0 = We, row1 = mean*We
            WeM_t = lw.tile([2, HC], BF16, tag="WeM")
            nc.sync.dma_start(out=WeM_t[:1, :], in_=We[l][:])
            meanWe = lw.tile([1, HC], BF16, tag="meanWe")
            nc.vector.tensor_scalar_mul(
                out=meanWe[:], in0=WeM_t[:1, :], scalar1=mean_t[:1, :1]
            )
            nc.sync.dma_start(out=WeM_t[1:2, :], in_=meanWe[:])

            # ---- BN params for this layer's input (from gathered features) ----
            if l > 0:
                with tc.tile_pool(name="stats", bufs=3) as stp:
                    s1cols = stp.tile([128, 24], F32, tag="s1c")
                    s2cols = stp.tile([128, 24], F32, tag="s2c")
                    CHK = NPC // 3 if NPC % 3 == 0 else NPC
                    nchk = NPC // CHK
                    idx = 0
                    for c8 in range(NCORES):
                        for j in range(nchk):
                            seg = stp.tile([128, CHK], BF16, tag="seg")
                            nc.sync.dma_start(
                                out=seg[:],
                                in_=ftg_dram[
                                    c8 * C : (c8 + 1) * C, j * CHK : (j + 1) * CHK
                                ],
                            )
                            nc.vector.reduce_sum(
                                out=s1cols[:, idx : idx + 1], in_=seg[:], axis=AX.X
                            )
                            sqt = stp.tile([128, CHK], F32, tag="sqt")
                            nc.scalar.activation(
                                out=sqt[:],
                                in_=seg[:],
                                func=ACT.Square,
                                accum_out=s2cols[:, idx : idx + 1],
                            )
                            idx += 1
                    S1 = bnp.tile([128, 1], F32, tag="bn_S1")
                    nc.vector.reduce_sum(out=S1[:], in_=s1cols[:, :idx], axis=AX.X)
                    S2 = bnp.tile([128, 1], F32, tag="bn_S2")
                    nc.vector.reduce_sum(out=S2[:], in_=s2cols[:, :idx], axis=AX.X)
                sc, bi, sc001, bi001 = emit_bn_params(S1[:, :1], S2[:, :1], l)

            # ---- projection phase ----
            xr_sb = persist.tile([128, NW * HC], BF16, tag="xr")

            def emit_bn_lhsT(pool, raw_ap, nn):
                """BN + leaky(0.01) fused input transform -> bf16 lhsT tile."""
                t1 = pool.tile([128, WIN], BF16, tag="bn_t1")
                nc.scalar.activation(
                    out=t1[:, :nn],
                    in_=raw_ap,
                    func=ACT.Identity,
                    scale=sc[:, :1],
                    bias=bi[:, :1],
                )
                t2 = pool.tile([128, WIN], BF16, tag="bn_t2")
                nc.scalar.activation(
                    out=t2[:, :nn],
                    in_=raw_ap,
                    func=ACT.Identity,
                    scale=sc001[:, :1],
                    bias=bi001[:, :1],
                )
                t3 = pool.tile([128, WIN], BF16, tag="bn_t3")
                nc.vector.tensor_tensor(
                    out=t3[:, :nn], in0=t1[:, :nn], in1=t2[:, :nn], op=OP.max
                )
                return t3

            with (
                tc.tile_pool(name="proj", bufs=3) as proj,
                tc.tile_pool(name="projps", bufs=2, space="PSUM") as projps,
            ):
                # xl for all nodes -> xl_dram
                for c8 in range(NCORES):
                    for w in range(NW):
                        nbase = c8 * NPC + w * WIN
                        nn = min(WIN, NPC - w * WIN)
                        if l == 0:
                            lh = proj.tile([IN, WIN], BF16, tag="lh0")
                            nc.sync.dma_start(
                                out=lh[:, :nn], in_=hT[:, nbase : nbase + nn]
                            )
                            lhsT_ap = lh[:, :nn]
                        else:
                            raw = proj.tile([128, WIN], BF16, tag="rawg")
                            nc.sync.dma_start(
                                out=raw[:, :nn],
                                in_=ftg_dram[
                                    c8 * C : (c8 + 1) * C,
                                    w * WIN : w * WIN + nn,
                                ],
                            )
                            lhsT_ap = emit_bn_lhsT(proj, raw[:, :nn], nn)[:, :nn]
                        ps = projps.tile([128, HC], F32, tag="xlps")
                        nc.tensor.matmul(
                            out=ps[:nn, :], lhsT=lhsT_ap, rhs=Wl_t[:], start=True, stop=False
                        )
                        nc.tensor.matmul(
                            out=ps[:nn, :],
                            lhsT=onesrow_t[:1, :nn],
                            rhs=bl_t[:],
                            start=False,
                            stop=True,
                        )
                        xlsb = proj.tile([128, HC], BF16, tag="xlsb")
                        nc.scalar.activation(out=xlsb[:nn, :], in_=ps[:nn, :], func=ACT.Copy)
                        nc.sync.dma_start(
                            out=xl_dram[nbase : nbase + nn, :], in_=xlsb[:nn, :]
                        )
                # xr for this core's nodes -> xr_sb
                for w in range(NW):
                    nn = min(WIN, NPC - w * WIN)
                    if l == 0:
                        lh = proj.tile([IN, WIN], BF16, tag="lh0")
                        nc.sync.dma_start(
                            out=lh[:, :nn], in_=hTloc[:, w * WIN : w * WIN + nn]
                        )
                        lhsT_ap = lh[:, :nn]
                    else:
                        raw_ap = ftl_sb_prev[:, w * WIN : w * WIN + nn]
                        lhsT_ap = emit_bn_lhsT(proj, raw_ap, nn)[:, :nn]
                    ps = projps.tile([128, HC], F32, tag="xlps")
                    nc.tensor.matmul(
                        out=ps[:nn, :], lhsT=lhsT_ap, rhs=Wr_t[:], start=True, stop=False
                    )
                    nc.tensor.matmul(
                        out=ps[:nn, :],
                        lhsT=onesrow_t[:1, :nn],
                        rhs=br_t[:],
                        start=False,
                        stop=True,
                    )
                    if nn < WIN:
                        nc.vector.memset(xr_sb[:, w * HC : (w + 1) * HC], 0.0)
                    nc.scalar.activation(
                        out=xr_sb[:nn, w * HC : (w + 1) * HC], in_=ps[:nn, :], func=ACT.Copy
                    )

            # ---- edge phase ----
            if not do_edge:
                break
            ftl_sb = persist.tile([C, NPC], BF16, tag="ftl")
            with (
                tc.tile_pool(name="edg", bufs=4) as edg,
                tc.tile_pool(name="edg2", bufs=3) as edg2,
                tc.tile_pool(name="ups", bufs=2, space="PSUM") as ups,
                tc.tile_pool(name="tps", bufs=2, space="PSUM") as tps,
                tc.tile_pool(name="winps", bufs=1, space="PSUM") as winps,
                tc.tile_pool(
                    name="statps", bufs=1, space="PSUM"
                ) as statps,
            ):
                if l == 2:
                    stat1_ps = statps.tile([128, 1], F32, tag="st1")
                    stat2_ps = statps.tile([128, 1], F32, tag="st2")
                for w in range(NW):
                    nn = min(WIN, NPC - w * WIN)
                    ncw = nch[w]
                    c0 = ch_off[w]
                    eam_t = edg.tile([2, max(nch) * 128], BF16, tag="eam")
                    nc.sync.dma_start(
                        out=eam_t[:, : ncw * 128],
                        in_=eam_in[:, c0 * 128 : (c0 + ncw) * 128],
                    )
                    pw0 = winps.tile([128, HC // 2 + 1], F32, tag="pw0")
                    pw1 = winps.tile([128, HC // 2 + 1], F32, tag="pw1")
                    for ci in range(ncw):
                        ch = c0 + ci
                        # gather xl[src]
                        xlg = edg.tile([128, HC], BF16, tag="xlg")
                        nc.gpsimd.indirect_dma_start(
                            out=xlg[:],
                            out_offset=None,
                            in_=xl_dram[:],
                            in_offset=bass.IndirectOffsetOnAxis(
                                ap=src_t[:, ch : ch + 1], axis=0
                            ),
                        )
                        if eparts < 2:
                            nc.vector.tensor_copy(
                                out=ftl_sb[:, :WIN], in_=xlg[:, :WIN]
                            )
                            continue
                        # indicator I[e, n] = (dst_rel[e] == n)
                        I_t = edg.tile([128, 128], BF16, tag="I")
                        nc.vector.tensor_scalar(
                            out=I_t[:],
                            in0=iota_t[:],
                            scalar1=dst_t[:, ch : ch + 1],
                            scalar2=None,
                            op0=OP.is_equal,
                        )
                        IT_ps = tps.tile([128, 128], BF16, tag="itps")
                        nc.tensor.transpose(
                            out=IT_ps[:], in_=I_t[:], identity=ident_t[:]
                        )
                        IT_sb = edg.tile([128, 128], BF16, tag="ITsb")
                        nc.vector.tensor_copy(out=IT_sb[:], in_=IT_ps[:])
                        if eparts < 3:
                            nc.vector.tensor_copy(out=ftl_sb[:, :WIN], in_=IT_sb[:])
                            continue
                        # u = xr[dst] + ea*We + xl[src]  (PSUM accumulation)
                        u_ps = ups.tile([128, HC], F32, tag="ups")
                        nc.tensor.matmul(
                            out=u_ps[:],
                            lhsT=IT_sb[:],
                            rhs=xr_sb[:, w * HC : (w + 1) * HC],
                            start=True,
                            stop=False,
                        )
                        nc.tensor.matmul(
                            out=u_ps[:],
                            lhsT=eam_t[:2, ci * 128 : (ci + 1) * 128],
                            rhs=WeM_t[:],
                            start=False,
                            stop=False,
                        )
                        nc.tensor.matmul(
                            out=u_ps[:],
                            lhsT=ident_t[:],
                            rhs=xlg[:],
                            start=False,
                            stop=True,
                        )
                        if eparts < 3.1:
                            nc.vector.tensor_copy(out=ftl_sb[:, :WIN], in_=u_ps[:, :WIN])
                            continue
                        # z = leaky_relu(u, 0.2); logit_h = sum z*att_h
                        t02 = edg2.tile([128, HC], F32, tag="t02")
                        nc.scalar.activation(
                            out=t02[:], in_=u_ps[:], func=ACT.Copy, scale=0.2
                        )
                        if eparts < 3.3:
                            nc.vector.tensor_copy(out=ftl_sb[:, :WIN], in_=t02[:, :WIN])
                            continue
                        z_t = edg2.tile([128, HC], F32, tag="z")
                        nc.vector.tensor_tensor(
                            out=z_t[:], in0=u_ps[:], in1=t02[:], op=OP.max
                        )
                        if eparts < 3.6:
                            nc.vector.tensor_copy(out=ftl_sb[:, :WIN], in_=z_t[:, :WIN])
                            continue
                        zw = edg2.tile([128, HC], F32, tag="zw")
                        logit2 = edg.tile([128, 2], F32, tag="logit")
                        nc.gpsimd.tensor_mul(out=zw[:], in0=z_t[:], in1=att_t[:])
                        for h in range(H):
                            nc.vector.reduce_sum(
                                out=logit2[:, h : h + 1],
                                in_=zw[:, h * C : (h + 1) * C],
                                axis=AX.X,
                            )
                        if eparts < 3.9:
                            nc.vector.tensor_copy(out=ftl_sb[:, :2], in_=logit2[:])
                            continue
                        q2 = edg.tile([128, 2], F32, tag="q2")
                        nc.scalar.activation(out=q2[:], in_=logit2[:], func=ACT.Exp)
                        if eparts < 5:
                            nc.vector.tensor_copy(out=ftl_sb[:, :WIN], in_=z_t[:, :WIN])
                            continue
                        # aggregate
                        for h, pw in ((0, pw0), (1, pw1)):
                            xlq = edg.tile([128, C + 1], BF16, tag=f"xlq{h}")
                            nc.scalar.activation(
                                out=xlq[:, :C],
                                in_=xlg[:, h * C : (h + 1) * C],
                                func=ACT.Identity,
                                scale=q2[:, h : h + 1],
                            )
                            nc.scalar.activation(
                                out=xlq[:, C : C + 1],
                                in_=q2[:, h : h + 1],
                                func=ACT.Copy,
                            )
                            nc.tensor.matmul(
                                out=pw[:],
                                lhsT=I_t[:],
                                rhs=xlq[:],
                                start=(ci == 0),
                                stop=(ci == ncw - 1),
                            )
                    if eparts < 5:
                        continue
                    # ---- window close ----
                    feat = edg2.tile([128, C], F32, tag="feat")
                    fh = []
                    for h, pw in ((0, pw0), (1, pw1)):
                        se = edg.tile([128, 1], F32, tag=f"se{h}")
                        nc.vector.tensor_scalar(
                            out=se[:],
                            in0=pw[:, C : C + 1],
                            scalar1=1e-10,
                            scalar2=None,
                            op0=OP.add,
                        )
                        rs = edg.tile([128, 1], F32, tag=f"rs{h}")
                        nc.vector.reciprocal(out=rs[:], in_=se[:])
                        f_h = edg2.tile([128, C], F32, tag=f"f{h}")
                        nc.scalar.activation(
                            out=f_h[:], in_=pw[:, :C], func=ACT.Identity, scale=rs[:, :1]
                        )
                        fh.append(f_h)
                    fsum = edg2.tile([128, C], F32, tag="fsum")
                    nc.vector.tensor_add(out=fsum[:], in0=fh[0][:], in1=fh[1][:])
                    fscaled = edg2.tile([128, C], F32, tag="fscaled")
                    nc.scalar.activation(
                        out=fscaled[:],
                        in_=fsum[:],
                        func=ACT.Identity,
                        scale=r2d_t[:, w : w + 1],
                    )
                    nc.vector.tensor_add(out=feat[:], in0=fscaled[:], in1=brep_t[:])
                    if nn < WIN:
                        featm = edg2.tile([128, C], F32, tag="featm")
                        nc.vector.tensor_scalar_mul(
                            out=featm[:], in0=feat[:], scalar1=lastmask_t[:, :1]
                        )
                        feat = featm
                    featbf = edg2.tile([128, C], BF16, tag="featbf")
                    nc.vector.tensor_copy(out=featbf[:], in_=feat[:])
                    if l == 2:
                        sq = edg2.tile([128, C], BF16, tag="sql2")
                        nc.scalar.activation(out=sq[:], in_=featbf[:], func=ACT.Square)
                        nc.tensor.matmul(
                            out=stat1_ps[:],
                            lhsT=featbf[:],
                            rhs=onescol_t[:],
                            start=(w == 0),
                            stop=(w == NW - 1),
                        )
                        nc.tensor.matmul(
                            out=stat2_ps[:],
                            lhsT=sq[:],
                            rhs=onescol_t[:],
                            start=(w == 0),
                            stop=(w == NW - 1),
                        )
                    fT_ps = tps.tile([128, 128], BF16, tag="itps")
                    nc.tensor.transpose(
                        out=fT_ps[:], in_=featbf[:], identity=ident_t[:]
                    )
                    nc.scalar.activation(
                        out=ftl_sb[:, w * WIN : w * WIN + nn],
                        in_=fT_ps[:, :nn],
                        func=ACT.Copy,
                    )
                    if l < 2:
                        nc.sync.dma_start(
                            out=ftl_dram[:, w * WIN : w * WIN + nn],
                            in_=ftl_sb[:, w * WIN : w * WIN + nn],
                        )
                if l == 2:
                    statsb = edg.tile([128, 2], F32, tag="statsb")
                    nc.scalar.activation(out=statsb[:, :1], in_=stat1_ps[:], func=ACT.Copy)
                    nc.scalar.activation(out=statsb[:, 1:2], in_=stat2_ps[:], func=ACT.Copy)
                    nc.sync.dma_start(out=arin[:], in_=statsb[:])

            # ---- collectives ----
            if not do_cc:
                ftl_sb_prev = ftl_sb
                continue
            if l < 2:
                nc.gpsimd.collective_compute(
                    "AllGather",
                    OP.bypass,
                    replica_groups=RG,
                    ins=[ftl_dram[:]],
                    outs=[ftg_dram[:]],
                )
            else:
                nc.gpsimd.collective_compute(
                    "AllReduce",
                    OP.add,
                    replica_groups=RG,
                    ins=[arin[:]],
                    outs=[arout[:]],
                )
            ftl_sb_prev = ftl_sb

        # ---- final FC over local nodes ----
        if do_fc:
          with (
            tc.tile_pool(name="fc", bufs=3) as fc,
            tc.tile_pool(name="fcps", bufs=2, space="PSUM") as fcps,
          ):
            stats_t = fc.tile([128, 2], F32, tag="arst")
            nc.gpsimd.dma_start(out=stats_t[:], in_=arout[:])
            sc, bi, sc001, bi001 = emit_bn_params(stats_t[:, 0:1], stats_t[:, 1:2], 2)
            for w in range(cfg.NW):
                nn = min(WIN, NPC - w * WIN)
                raw_ap = ftl_sb_prev[:, w * WIN : w * WIN + nn]
                t1 = fc.tile([128, WIN], BF16, tag="fc_t1")
                nc.scalar.activation(
                    out=t1[:, :nn], in_=raw_ap, func=ACT.Identity,
                    scale=sc[:, :1], bias=bi[:, :1],
                )
                t2 = fc.tile([128, WIN], BF16, tag="fc_t2")
                nc.scalar.activation(
                    out=t2[:, :nn], in_=raw_ap, func=ACT.Identity,
                    scale=sc001[:, :1], bias=bi001[:, :1],
                )
                t3 = fc.tile([128, WIN], BF16, tag="fc_t3")
                nc.vector.tensor_tensor(
                    out=t3[:, :nn], in0=t1[:, :nn], in1=t2[:, :nn], op=OP.max
                )
                ps = fcps.tile([128, OUT], F32, tag="fcps")
                nc.tensor.matmul(
                    out=ps[:nn, :], lhsT=t3[:, :nn], rhs=fcW_t[:], start=True, stop=False
                )
                nc.tensor.matmul(
                    out=ps[:nn, :],
                    lhsT=onesrow_t[:1, :nn],
                    rhs=fcb_t[:],
                    start=False,
                    stop=True,
                )
                osb = fc.tile([128, OUT], F32, tag="osb")
                nc.scalar.activation(out=osb[:nn, :], in_=ps[:nn, :], func=ACT.Copy)
                nc.sync.dma_start(
                    out=out_ext[w * WIN : w * WIN + nn, :], in_=osb[:nn, :]
                )

        for p in (bnp, persist, lw, const):
            p.release()

    nc.finalize()
    return nc


def make_in_maps(cfg, inputs, pre):
    """Build the 8 per-core input maps from full problem inputs + preprocess data."""
    N, E, IN, OUT = cfg.N, cfg.E, cfg.IN, cfg.OUT
    NPC, NW = cfg.NPC, cfg.NW

    h = np.asarray(inputs["h"], NPF32)
    hT = np.ascontiguousarray(h.T).astype(NPBF16)
    ew_tile = (
        np.asarray(inputs["edge_weight"], NPF32).reshape(-1).reshape(128, cfg.EW_COLS, order="F")
    )
    # order="F": ew[p, c] = w[c*128+p]; any order works for a sum
    iota_np = np.broadcast_to(np.arange(128, dtype=NPF32)[None, :], (128, 128)).copy()
    ident_np = np.eye(128, dtype=NPBF16)
    onesrow_np = np.ones((1, 128), NPBF16)
    onescol_np = np.ones((128, 1), NPBF16)
    lastn = NPC - (NW - 1) * WIN
    lastmask_np = (np.arange(128) < lastn).astype(NPF32).reshape(128, 1)

    common = {
        "hT": hT,
        "ew": ew_tile.astype(NPF32),
        "iota": iota_np,
        "identbf": ident_np,
        "onesrow": onesrow_np,
        "onescol": onescol_np,
        "lastmask": lastmask_np,
        "fcW": np.asarray(inputs["fcW"], NPF32).astype(NPBF16),
        "fcb": np.asarray(inputs["fcb"], NPF32).reshape(1, OUT).astype(NPBF16),
    }
    for l in range(3):
        att = np.asarray(inputs[f"att{l}"], NPF32).reshape(1, H * C)
        common[f"Wl{l}"] = np.asarray(inputs[f"Wl{l}"], NPF32).astype(NPBF16)
        common[f"bl{l}"] = np.asarray(inputs[f"bl{l}"], NPF32).reshape(1, H * C).astype(NPBF16)
        common[f"Wr{l}"] = np.asarray(inputs[f"Wr{l}"], NPF32).astype(NPBF16)
        common[f"br{l}"] = np.asarray(inputs[f"br{l}"], NPF32).reshape(1, H * C).astype(NPBF16)
        common[f"We{l}"] = np.asarray(inputs[f"We{l}"], NPF32).astype(NPBF16)
        common[f"attrep{l}"] = np.broadcast_to(att, (128, H * C)).copy()
        common[f"brep{l}"] = np.broadcast_to(
            np.asarray(inputs[f"b{l}"], NPF32)[None, :], (128, C)
        ).copy()
        common[f"g{l}"] = np.asarray(inputs[f"g{l}"], NPF32).reshape(128, 1)
        common[f"bt{l}"] = np.asarray(inputs[f"bt{l}"], NPF32).reshape(128, 1)

    in_maps = []
    for k in range(NCORES):
        m = dict(common)
        m["hTloc"] = np.ascontiguousarray(hT[:, k * NPC : (k + 1) * NPC])
        m["srccols"] = pre["src_cols"][k]
        m["dstcols"] = pre["dst_cols"][k]
        m["eam"] = pre["eam"][k].astype(NPBF16)
        m["r2d"] = pre["r2d"][k]
        in_maps.append(m)
    return in_maps


_CACHE = {}


def kernel(**inputs) -> np.ndarray:
    from concourse.bass_utils import run_bass_kernel_spmd

    cfg = Cfg(N=30000, E=480000, IN=6, OUT=5)
    pre = preprocess(
        cfg,
        np.asarray(inputs["edge_index"], np.int64),
        np.asarray(inputs["edge_weight"], NPF32),
    )
    key = ("prog", tuple(pre["nch"]))
    if key not in _CACHE:
        _CACHE[key] = build_program(cfg, pre["nch"], pre["ch_off"], pre["totch"])
    nc = _CACHE[key]
    in_maps = make_in_maps(cfg, inputs, pre)
    res = run_bass_kernel_spmd(nc, in_maps, core_ids=list(range(NCORES)))
    out = np.concatenate([res.results[k]["out"] for k in range(NCORES)], axis=0)
    return out.astype(NPF32)
